# revision 20
# baseline (speedup 1.0000x reference)
"""ComplexAttention Trainium2 kernel (8 NeuronCores, Bass/Tile).

Problem: complex-valued QKV projections + causal attention, B=4, S=2048, D=1024.
  qr,qi / kr,ki / vr,vi = complex_linear(z, W*)          (z @ W^T per component)
  scores = (qr@kr^T + qi@ki^T) / sqrt(D), causal mask, softmax
  out = stack([attn@vr, attn@vi])                        -> [2, B, S, D]

Sharding (uniform SPMD, 8 cores): core c -> (batch b = c//2, d-half = c%2).
Each core computes the q/k projections for its batch restricted to its
512-wide dout half (weights arrive host-sliced), then a per-512-seq-chunk
packed AllGather within the batch pair assembles full q^T/k^T — the gathers
pipeline with the remaining projection work. The v projection covers only
the core's d-half and stays resident in SBUF (no DRAM roundtrip). Every core
computes full causal softmax statistics for its batch and the attention
output for its d-half (scores/exp duplicated within the pair; the av
matmuls and v are split).

All matmul operands are bf16 (fp32 PSUM accumulation): this halves the
pair AllGather bytes (the previous fp32r gathers were the critical path),
halves HBM traffic, and enables fast-weight-load so LDWEIGHTS hides under
the matmul stream. Host pre-rounds the kernel inputs to bf16.

Math note: softmax is computed without max-subtraction (scores are O(10), so
exp() is safe), as exp(s/sqrt(D)) normalized by a ones-matmul denominator.
Masking multiplies exp by a 0/1 mask on the 4 diagonal key blocks of each
512-query macro block; off-diagonal blocks are either fully kept or skipped
entirely.
"""

import numpy as np

B, S, D = 4, 2048, 1024
P = 128
SQ = 512  # query macro-block width / matmul moving width
NDC = D // P  # 8 contraction chunks
NM = S // SQ  # 4 query macro blocks
NKB = S // P  # 16 key blocks
N_CORES = 8
SCALE = float(D) ** -0.5

_COMPILED = {}


def _to_bf16(x: np.ndarray) -> np.ndarray:
    import ml_dtypes

    return np.ascontiguousarray(np.asarray(x, dtype=np.float32)).astype(
        ml_dtypes.bfloat16
    )


def _to_f16(x: np.ndarray) -> np.ndarray:
    return np.ascontiguousarray(np.asarray(x, dtype=np.float32)).astype(np.float16)


def _build_module(reps: int = 1):
    import concourse.tile as tile
    from concourse import bacc, mybir

    f32 = mybir.dt.float32
    f16 = mybir.dt.float16
    bf16 = mybir.dt.bfloat16
    EXP = mybir.ActivationFunctionType.Exp
    COPY = mybir.ActivationFunctionType.Copy
    AX = mybir.AxisListType.X

    nc = bacc.Bacc("TRN2", target_bir_lowering=False, debug=False, num_devices=8)

    # ---- I/O ----
    # q/k/v projection weights arrive pre-sliced to this core's dout half.
    zr_d = nc.dram_tensor("zr", [D, S], f16, kind="ExternalInput")
    zi_d = nc.dram_tensor("zi", [D, S], f16, kind="ExternalInput")
    wq_r = nc.dram_tensor("wqr", [D, SQ], f16, kind="ExternalInput")
    wq_i = nc.dram_tensor("wqi", [D, SQ], f16, kind="ExternalInput")
    wk_r = nc.dram_tensor("wkr", [D, SQ], f16, kind="ExternalInput")
    wk_i = nc.dram_tensor("wki", [D, SQ], f16, kind="ExternalInput")
    wv_r = nc.dram_tensor("wvr", [D, SQ], f16, kind="ExternalInput")
    wv_i = nc.dram_tensor("wvi", [D, SQ], f16, kind="ExternalInput")
    dmask_d = nc.dram_tensor("dmask", [4, P, P], bf16, kind="ExternalInput")
    o_d = nc.dram_tensor("o", [2, S, SQ], f32, kind="ExternalOutput")

    # ---- DRAM scratch ----
    # q/k halves per 512-seq chunk, packed [qk, ci, 512, 512] so one
    # AllGather per chunk moves everything; gathered adds a leading pair
    # -half axis g: qkTs[sc][g, qk, ci, row, s] with dout = g*512 + row.
    qkTsh = nc.dram_tensor("qkTsh", [NM, 2, 2, SQ, SQ], f16, kind="Internal")
    qkTs = nc.dram_tensor("qkTs", [NM, 2, 2, 2, SQ, SQ], f16, kind="Internal")
    PAIRS = [[0, 1], [2, 3], [4, 5], [6, 7]]

    def load_z_chunk(zp, sc):
        """Load z^T tiles for one 512-seq chunk: (r, i) x 8 dim-chunks."""
        ztr, zti = [], []
        for dc in range(NDC):
            tr = zp.tile([P, SQ], f16, tag="zt", name=f"ztr{dc}")
            nc.sync.dma_start(
                tr[:], zr_d[dc * P : (dc + 1) * P, sc * SQ : (sc + 1) * SQ]
            )
            ti = zp.tile([P, SQ], f16, tag="zt", name=f"zti{dc}")
            nc.sync.dma_start(
                ti[:], zi_d[dc * P : (dc + 1) * P, sc * SQ : (sc + 1) * SQ]
            )
            ztr.append(tr)
            zti.append(ti)
        return ztr, zti

    def load_w_chunks(wp, w_d, width, nm):
        """Load a weight matrix as 8 per-dc tiles [P, width] (one DMA each)."""
        tiles = []
        for dc in range(NDC):
            t = wp.tile([P, SQ], f16, tag="w", name=f"{nm}{dc}")
            nc.sync.dma_start(t[:, 0:width], w_d[dc * P : (dc + 1) * P, 0:width])
            tiles.append(t)
        return tiles

    def load_plane(t, sc, qk, ci, eng):
        """Load one gathered 512-seq plane as [P, (dc, s)] with dc = g*4+c."""
        for g in range(2):
            eng.dma_start(
                t[:, g * 4 * SQ : (g + 1) * 4 * SQ].rearrange(
                    "p (c q) -> p c q", c=4
                ),
                qkTs[sc, g, qk, ci].rearrange("(c p) q -> p c q", p=P),
            )

    def emit_projections(v_tiles, ktpl, kp):
        with (
            tc.tile_pool(name="wpool", bufs=48) as wp,
            tc.tile_pool(name="zpool", bufs=36) as zp,
            tc.tile_pool(name="stg", bufs=4) as sp,
            tc.tile_pool(name="ppsum", bufs=6, space="PSUM") as pp,
        ):
            # ---- q and k projections (full S, this core's dout half) ----
            # q and k share each z chunk; one packed AllGather per chunk
            # pipelines the pair exchange with the remaining projection work.
            wq_t = (load_w_chunks(wp, wq_r, SQ, "wqr"), load_w_chunks(wp, wq_i, SQ, "wqi"))
            wk_t = (load_w_chunks(wp, wk_r, SQ, "wkr"), load_w_chunks(wp, wk_i, SQ, "wki"))
            wvr_t = load_w_chunks(wp, wv_r, SQ, "wvr")
            wvi_t = load_w_chunks(wp, wv_i, SQ, "wvi")
            for sc in range(NM):
                ztr, zti = load_z_chunk(zp, sc)
                for qk, (wr_t, wi_t) in enumerate((wq_t, wk_t)):
                    for db in range(SQ // P):
                        # real: (z_r@w_r) - (z_i@w_i) as a PSUM-pair
                        # difference (no negated-z tiles to keep live).
                        psa = pp.tile([P, SQ], f32, tag="pp", name="psa")
                        psb = pp.tile([P, SQ], f32, tag="pp", name="psb")
                        for n in range(NDC):
                            nc.tensor.matmul(
                                psa[:],
                                wr_t[n][:, db * P : (db + 1) * P],
                                ztr[n][:],
                                start=(n == 0),
                                stop=(n == NDC - 1),
                            )
                        for n in range(NDC):
                            nc.tensor.matmul(
                                psb[:],
                                wi_t[n][:, db * P : (db + 1) * P],
                                zti[n][:],
                                start=(n == 0),
                                stop=(n == NDC - 1),
                            )
                        st = sp.tile([P, SQ], f16, tag="st", name="st")
                        nc.scalar.activation(st[:], psb[:], COPY, scale=-1.0)
                        nc.vector.tensor_add(st[:], psa[:], st[:])
                        nc.sync.dma_start(
                            qkTsh[sc, qk, 0, db * P : (db + 1) * P, :], st[:]
                        )
                        # imag: z_i@w_r + z_r@w_i accumulates in one bank
                        ps = pp.tile([P, SQ], f32, tag="pp", name="ps")
                        n = 0
                        for w_t, zt in ((wr_t, zti), (wi_t, ztr)):
                            for dc in range(NDC):
                                nc.tensor.matmul(
                                    ps[:],
                                    w_t[dc][:, db * P : (db + 1) * P],
                                    zt[dc][:],
                                    start=(n == 0),
                                    stop=(n == 15),
                                )
                                n += 1
                        st = sp.tile([P, SQ], f16, tag="st", name="st")
                        nc.vector.tensor_copy(st[:], ps[:])
                        nc.sync.dma_start(
                            qkTsh[sc, qk, 1, db * P : (db + 1) * P, :], st[:]
                        )
                nc.gpsimd.collective_compute(
                    "AllGather",
                    mybir.AluOpType.bypass,
                    replica_groups=PAIRS,
                    ins=[qkTsh[sc].opt()],
                    outs=[qkTs[sc].opt()],
                )
                # k^T plane for this chunk (and q^T for the first two
                # macros) load on the Scalar engine's HWDGE queue as soon
                # as the gather lands — the Sync queue keeps draining
                # projection DMAs without head-of-line blocking.
                for ci in range(2):
                    t = kp.tile([P, NDC * SQ], f16, tag="ktch", name=f"kt{sc}{ci}")
                    load_plane(t, sc, 1, ci, nc.scalar)
                    ktpl[(sc, ci)] = t

                # ---- v projection for this chunk (shares the z tiles) ----
                # v[kb] = [128 keys, (vr 512 | vi 512)] stays resident in SBUF.
                for sbl in range(SQ // P):
                    lo, hi = sbl * P, (sbl + 1) * P
                    kb = sc * 4 + sbl
                    psa = pp.tile([P, SQ], f32, tag="pp", name="psa")
                    psb = pp.tile([P, SQ], f32, tag="pp", name="psb")
                    for n in range(NDC):
                        nc.tensor.matmul(
                            psa[:],
                            ztr[n][:, lo:hi],
                            wvr_t[n][:, 0:SQ],
                            start=(n == 0),
                            stop=(n == NDC - 1),
                        )
                    for n in range(NDC):
                        nc.tensor.matmul(
                            psb[:],
                            zti[n][:, lo:hi],
                            wvi_t[n][:, 0:SQ],
                            start=(n == 0),
                            stop=(n == NDC - 1),
                        )
                    nc.scalar.activation(v_tiles[kb][:, 0:SQ], psb[:], COPY, scale=-1.0)
                    nc.vector.tensor_add(
                        v_tiles[kb][:, 0:SQ], psa[:], v_tiles[kb][:, 0:SQ]
                    )
                    ps = pp.tile([P, SQ], f32, tag="pp", name="ps")
                    n = 0
                    for zt, w_t in ((zti, wvr_t), (ztr, wvi_t)):
                        for dc in range(NDC):
                            nc.tensor.matmul(
                                ps[:],
                                zt[dc][:, lo:hi],
                                w_t[dc][:, 0:SQ],
                                start=(n == 0),
                                stop=(n == 15),
                            )
                            n += 1
                    nc.vector.tensor_copy(
                        v_tiles[kb][:, SQ : 2 * SQ], ps[:]
                    )

    def emit_attention(mask_t, ones_t, v_tiles, ktpl):
        qtt = {}
        with (
            tc.tile_pool(name="qtpl", bufs=2) as qp,
            tc.tile_pool(name="expp", bufs=20) as ep,
            tc.tile_pool(name="outp", bufs=4) as op,
            tc.tile_pool(name="smal", bufs=2) as smp,
            tc.tile_pool(name="spsum", bufs=2, space="PSUM") as sps,
            tc.tile_pool(name="dpsum", bufs=1, space="PSUM") as dps,
            tc.tile_pool(name="avpsum", bufs=5, space="PSUM") as avp,
        ):
            for m in range(NM):
                nkb = 4 * (m + 1)  # causal key blocks for this macro
                jm = 4 * m  # first diagonal key block
                if (m, 0) not in qtt:
                    for ci in range(2):
                        t = qp.tile(
                            [P, NDC * SQ], f16, tag=f"qt{ci}", name=f"qt{m}{ci}"
                        )
                        load_plane(t, m, 0, ci, nc.sync)
                        qtt[(m, ci)] = t
                qt = [qtt[(m, 0)], qtt[(m, 1)]]
                den_ps = dps.tile([P, 64], f32, tag="den", name="den_ps")
                expts = []
                for kb in range(nkb):
                    # diagonal blocks: only queries >= the block's own row
                    # range are causally valid; trim the moving operand.
                    qo = (kb - jm) * P if kb >= jm else 0
                    sc, toff = kb // 4, (kb % 4) * P
                    ps = sps.tile([P, SQ], f32, tag="sc", name="ps")
                    n = 0
                    for ci in range(2):
                        for dc in range(NDC):
                            nc.tensor.matmul(
                                ps[:, qo:SQ],
                                ktpl[(sc, ci)][:, dc * SQ + toff : dc * SQ + toff + P],
                                qt[ci][:, dc * SQ + qo : (dc + 1) * SQ],
                                start=(n == 0),
                                stop=(n == 15),
                            )
                            n += 1
                    et = ep.tile([P, SQ], bf16, tag="et", name="et")
                    nc.scalar.activation(et[:, qo:SQ], ps[:, qo:SQ], EXP, scale=SCALE)
                    if kb >= jm:
                        sub = kb - jm
                        nc.vector.tensor_mul(
                            et[:, sub * P : (sub + 1) * P],
                            et[:, sub * P : (sub + 1) * P],
                            mask_t[sub][:],
                        )
                    expts.append(et)
                    for sub in range(max(0, kb - jm), 4):
                        c = sub * 16 + kb
                        nc.tensor.matmul(
                            den_ps[:, c : c + 1],
                            et[:, sub * P : (sub + 1) * P],
                            ones_t[:],
                            start=True,
                            stop=True,
                        )
                den_sb = smp.tile([P, 4], f32, tag="densb", name="den_sb")
                for sub in range(4):
                    nc.vector.reduce_sum(
                        den_sb[:, sub : sub + 1],
                        den_ps[:, sub * 16 : sub * 16 + jm + sub + 1],
                        axis=AX,
                    )
                recip = smp.tile([P, 4], f32, tag="recip", name="recip")
                nc.vector.reciprocal(recip[:], den_sb[:])

                for pair in range(2):
                    subs = (2 * pair, 2 * pair + 1)
                    j_hi = jm + subs[1]
                    av = {}
                    for sl in range(2):
                        for ci in range(2):
                            av[(sl, ci)] = avp.tile(
                                [P, SQ], f32, tag="av", name=f"av{sl}{ci}"
                            )
                    for kb in range(j_hi + 1):
                        for sl, sub in enumerate(subs):
                            j = jm + sub
                            if kb > j:
                                continue
                            for ci in range(2):
                                nc.tensor.matmul(
                                    av[(sl, ci)][:],
                                    expts[kb][:, sub * P : (sub + 1) * P],
                                    v_tiles[kb][:, ci * SQ : (ci + 1) * SQ],
                                    start=(kb == 0),
                                    stop=(kb == j),
                                )
                    for sl, sub in enumerate(subs):
                        row = m * SQ + sub * P
                        for ci in range(2):
                            ot = op.tile([P, SQ], f32, tag="ot", name="ot")
                            nc.vector.tensor_scalar_mul(
                                ot[:], av[(sl, ci)][:], recip[:, sub : sub + 1]
                            )
                            nc.sync.dma_start(o_d[ci, row : row + P, :], ot[:])

    with tile.TileContext(nc) as tc:
        with tc.tile_pool(name="const", bufs=1) as cp:
            mask_t = []
            for idx in range(4):
                mt = cp.tile([P, P], bf16, tag=f"mask{idx}", name=f"mask{idx}")
                nc.sync.dma_start(mt[:], dmask_d[idx])
                mask_t.append(mt)
            ones_t = cp.tile([P, 1], bf16, tag="ones", name="ones_t")
            nc.vector.memset(ones_t[:], 1.0)
            for _rep in range(reps):
                with (
                    tc.tile_pool(name="vres", bufs=16) as vp,
                    tc.tile_pool(name="ktpl", bufs=8) as kp,
                ):
                    v_tiles = [
                        vp.tile([P, 2 * SQ], f16, tag="v", name=f"v{kb}")
                        for kb in range(NKB)
                    ]
                    ktpl = {}
                    emit_projections(v_tiles, ktpl, kp)
                    emit_attention(mask_t, ones_t, v_tiles, ktpl)

    nc.compile()
    return nc


def get_module(reps: int = 1):
    key = ("nc", reps)
    if key not in _COMPILED:
        _COMPILED[key] = _build_module(reps)
    return _COMPILED[key]


def prepare_in_maps(z_real, z_imag, wq_r, wq_i, wk_r, wk_i, wv_r, wv_i, mask):
    """Host-side sharding/layout prep -> list of per-core input dicts."""
    r = _to_f16
    zT_r = [r(np.asarray(z_real, np.float32)[b].T) for b in range(B)]
    zT_i = [r(np.asarray(z_imag, np.float32)[b].T) for b in range(B)]
    # weights: torch Linear W is [out, in]; matmuls want W^T = [in, out]
    wqr_T = r(np.asarray(wq_r).T)
    wqi_T = r(np.asarray(wq_i).T)
    wkr_T = r(np.asarray(wk_r).T)
    wki_T = r(np.asarray(wk_i).T)
    wvr_T = r(np.asarray(wv_r).T)
    wvi_T = r(np.asarray(wv_i).T)
    # diagonal-block masks from the provided mask (macro 3 as representative)
    mask = np.asarray(mask)
    dmask = np.zeros((4, P, P), np.float32)
    g0 = 3 * SQ
    for idx in range(4):
        k0 = (12 + idx) * P
        q0 = idx * P
        dmask[idx] = (
            mask[g0 + q0 : g0 + q0 + P, k0 : k0 + P] != 0
        ).T.astype(np.float32)
    dmask = _to_bf16(dmask)
    in_maps = []
    for c in range(N_CORES):
        b, dh = c // 2, c % 2
        half = slice(dh * SQ, (dh + 1) * SQ)
        in_maps.append(
            {
                "zr": zT_r[b],
                "zi": zT_i[b],
                "wqr": np.ascontiguousarray(wqr_T[:, half]),
                "wqi": np.ascontiguousarray(wqi_T[:, half]),
                "wkr": np.ascontiguousarray(wkr_T[:, half]),
                "wki": np.ascontiguousarray(wki_T[:, half]),
                "wvr": np.ascontiguousarray(wvr_T[:, half]),
                "wvi": np.ascontiguousarray(wvi_T[:, half]),
                "dmask": dmask,
            }
        )
    return in_maps


def assemble_output(results):
    """Per-core outputs [2, S, 512] -> full [2, B, S, D]."""
    out = np.empty((2, B, S, D), np.float32)
    for c in range(N_CORES):
        b, dh = c // 2, c % 2
        out[:, b, :, dh * SQ : (dh + 1) * SQ] = results[c]["o"]
    return out


def kernel(**inputs) -> np.ndarray:
    from concourse.bass_utils import run_bass_kernel_spmd

    nc = get_module()
    in_maps = prepare_in_maps(**inputs)
    res = run_bass_kernel_spmd(nc, in_maps, core_ids=list(range(N_CORES)))
    return assemble_output(res.results)


# revision 21
# speedup vs baseline: 1.0995x; 1.0995x over previous
"""ComplexAttention Trainium2 kernel (8 NeuronCores, Bass/Tile).

Problem: complex-valued QKV projections + causal attention, B=4, S=2048, D=1024.
  qr,qi / kr,ki / vr,vi = complex_linear(z, W*)          (z @ W^T per component)
  scores = (qr@kr^T + qi@ki^T) / sqrt(D), causal mask, softmax
  out = stack([attn@vr, attn@vi])                        -> [2, B, S, D]

Sharding (uniform SPMD, 8 cores): core c -> (batch b = c//2, d-half = c%2).
Each core computes the q/k projections for its batch restricted to its
512-wide dout half (weights arrive host-sliced), then a per-512-seq-chunk
packed AllGather within the batch pair assembles full q^T/k^T — the gathers
pipeline with the remaining projection work. The v projection covers only
the core's d-half and stays resident in SBUF (no DRAM roundtrip). Every core
computes full causal softmax statistics for its batch and the attention
output for its d-half (scores/exp duplicated within the pair; the av
matmuls and v are split).

All matmul operands are bf16 (fp32 PSUM accumulation): this halves the
pair AllGather bytes (the previous fp32r gathers were the critical path),
halves HBM traffic, and enables fast-weight-load so LDWEIGHTS hides under
the matmul stream. Host pre-rounds the kernel inputs to bf16.

Math note: softmax is computed without max-subtraction (scores are O(10), so
exp() is safe), as exp(s/sqrt(D)) normalized by a ones-matmul denominator.
Masking multiplies exp by a 0/1 mask on the 4 diagonal key blocks of each
512-query macro block; off-diagonal blocks are either fully kept or skipped
entirely.
"""

import numpy as np

B, S, D = 4, 2048, 1024
P = 128
SQ = 512  # query macro-block width / matmul moving width
NDC = D // P  # 8 contraction chunks
NM = S // SQ  # 4 query macro blocks
NKB = S // P  # 16 key blocks
N_CORES = 8
SCALE = float(D) ** -0.5

_COMPILED = {}


def _to_bf16(x: np.ndarray) -> np.ndarray:
    import ml_dtypes

    return np.ascontiguousarray(np.asarray(x, dtype=np.float32)).astype(
        ml_dtypes.bfloat16
    )


def _to_f16(x: np.ndarray) -> np.ndarray:
    return np.ascontiguousarray(np.asarray(x, dtype=np.float32)).astype(np.float16)


def _build_module(reps: int = 1):
    import concourse.tile as tile
    from concourse import bacc, mybir

    f32 = mybir.dt.float32
    f16 = mybir.dt.float16
    bf16 = mybir.dt.bfloat16
    EXP = mybir.ActivationFunctionType.Exp
    COPY = mybir.ActivationFunctionType.Copy
    AX = mybir.AxisListType.X

    nc = bacc.Bacc("TRN2", target_bir_lowering=False, debug=False, num_devices=8)

    # ---- I/O ----
    # q/k/v projection weights arrive pre-sliced to this core's dout half.
    zr_d = nc.dram_tensor("zr", [D, S], f16, kind="ExternalInput")
    zi_d = nc.dram_tensor("zi", [D, S], f16, kind="ExternalInput")
    wq_r = nc.dram_tensor("wqr", [D, SQ], f16, kind="ExternalInput")
    wq_i = nc.dram_tensor("wqi", [D, SQ], f16, kind="ExternalInput")
    wk_r = nc.dram_tensor("wkr", [D, SQ], f16, kind="ExternalInput")
    wk_i = nc.dram_tensor("wki", [D, SQ], f16, kind="ExternalInput")
    wv_r = nc.dram_tensor("wvr", [D, SQ], f16, kind="ExternalInput")
    wv_i = nc.dram_tensor("wvi", [D, SQ], f16, kind="ExternalInput")
    dmask_d = nc.dram_tensor("dmask", [4, P, P], bf16, kind="ExternalInput")
    o_d = nc.dram_tensor("o", [2, S, SQ], f32, kind="ExternalOutput")

    # ---- DRAM scratch ----
    # q/k halves per 512-seq chunk, packed [qk, ci, 512, 512] so one
    # AllGather per chunk moves everything; gathered adds a leading pair
    # -half axis g: qkTs[sc][g, qk, ci, row, s] with dout = g*512 + row.
    qkTsh = nc.dram_tensor("qkTsh", [NM, 2, 2, SQ, SQ], f16, kind="Internal")
    qkTs = nc.dram_tensor("qkTs", [NM, 2, 2, 2, SQ, SQ], f16, kind="Internal")
    PAIRS = [[0, 1], [2, 3], [4, 5], [6, 7]]

    def load_z_chunk(zp, sc):
        """Load z^T tiles for one 512-seq chunk: (r, i) x 8 dim-chunks."""
        ztr, zti = [], []
        for dc in range(NDC):
            tr = zp.tile([P, SQ], f16, tag="zt", name=f"ztr{dc}")
            nc.sync.dma_start(
                tr[:], zr_d[dc * P : (dc + 1) * P, sc * SQ : (sc + 1) * SQ]
            )
            ti = zp.tile([P, SQ], f16, tag="zt", name=f"zti{dc}")
            nc.sync.dma_start(
                ti[:], zi_d[dc * P : (dc + 1) * P, sc * SQ : (sc + 1) * SQ]
            )
            ztr.append(tr)
            zti.append(ti)
        return ztr, zti

    def load_w_chunks(wp, w_d, width, nm):
        """Load a weight matrix as 8 per-dc tiles [P, width] (one DMA each)."""
        tiles = []
        for dc in range(NDC):
            t = wp.tile([P, SQ], f16, tag="w", name=f"{nm}{dc}")
            nc.sync.dma_start(t[:, 0:width], w_d[dc * P : (dc + 1) * P, 0:width])
            tiles.append(t)
        return tiles

    def load_plane(t, sc, qk, ci, eng):
        """Load one gathered 512-seq plane as [P, (dc, s)] with dc = g*4+c."""
        for g in range(2):
            eng.dma_start(
                t[:, g * 4 * SQ : (g + 1) * 4 * SQ].rearrange(
                    "p (c q) -> p c q", c=4
                ),
                qkTs[sc, g, qk, ci].rearrange("(c p) q -> p c q", p=P),
            )

    def emit_projections(v_tiles, ktpl, kp):
        with (
            tc.tile_pool(name="wpool", bufs=48) as wp,
            tc.tile_pool(name="zpool", bufs=36) as zp,
            tc.tile_pool(name="stg", bufs=4) as sp,
            tc.tile_pool(name="ppsum", bufs=6, space="PSUM") as pp,
        ):
            # ---- q and k projections (full S, this core's dout half) ----
            # q and k share each z chunk; one packed AllGather per chunk
            # pipelines the pair exchange with the remaining projection work.
            wq_t = (load_w_chunks(wp, wq_r, SQ, "wqr"), load_w_chunks(wp, wq_i, SQ, "wqi"))
            wk_t = (load_w_chunks(wp, wk_r, SQ, "wkr"), load_w_chunks(wp, wk_i, SQ, "wki"))
            wvr_t = load_w_chunks(wp, wv_r, SQ, "wvr")
            wvi_t = load_w_chunks(wp, wv_i, SQ, "wvi")
            for sc in range(NM):
                ztr, zti = load_z_chunk(zp, sc)
                for qk, (wr_t, wi_t) in enumerate((wq_t, wk_t)):
                    for db in range(SQ // P):
                        # real: (z_r@w_r) - (z_i@w_i) as a PSUM-pair
                        # difference (no negated-z tiles to keep live).
                        psa = pp.tile([P, SQ], f32, tag="pp", name="psa")
                        psb = pp.tile([P, SQ], f32, tag="pp", name="psb")
                        for n in range(NDC):
                            nc.tensor.matmul(
                                psa[:],
                                wr_t[n][:, db * P : (db + 1) * P],
                                ztr[n][:],
                                start=(n == 0),
                                stop=(n == NDC - 1),
                            )
                        for n in range(NDC):
                            nc.tensor.matmul(
                                psb[:],
                                wi_t[n][:, db * P : (db + 1) * P],
                                zti[n][:],
                                start=(n == 0),
                                stop=(n == NDC - 1),
                            )
                        st = sp.tile([P, SQ], f16, tag="st", name="st")
                        nc.scalar.activation(st[:], psb[:], COPY, scale=-1.0)
                        nc.vector.tensor_add(st[:], psa[:], st[:])
                        nc.sync.dma_start(
                            qkTsh[sc, qk, 0, db * P : (db + 1) * P, :], st[:]
                        )
                        # imag: z_i@w_r + z_r@w_i accumulates in one bank
                        ps = pp.tile([P, SQ], f32, tag="pp", name="ps")
                        n = 0
                        for w_t, zt in ((wr_t, zti), (wi_t, ztr)):
                            for dc in range(NDC):
                                nc.tensor.matmul(
                                    ps[:],
                                    w_t[dc][:, db * P : (db + 1) * P],
                                    zt[dc][:],
                                    start=(n == 0),
                                    stop=(n == 15),
                                )
                                n += 1
                        st = sp.tile([P, SQ], f16, tag="st", name="st")
                        nc.vector.tensor_copy(st[:], ps[:])
                        nc.sync.dma_start(
                            qkTsh[sc, qk, 1, db * P : (db + 1) * P, :], st[:]
                        )
                nc.gpsimd.collective_compute(
                    "AllGather",
                    mybir.AluOpType.bypass,
                    replica_groups=PAIRS,
                    ins=[qkTsh[sc].opt()],
                    outs=[qkTs[sc].opt()],
                )

                # ---- v projection for this chunk (shares the z tiles) ----
                # v[kb] = [128 keys, (vr 512 | vi 512)] stays resident in SBUF.
                for sbl in range(SQ // P):
                    lo, hi = sbl * P, (sbl + 1) * P
                    kb = sc * 4 + sbl
                    psa = pp.tile([P, SQ], f32, tag="pp", name="psa")
                    psb = pp.tile([P, SQ], f32, tag="pp", name="psb")
                    for n in range(NDC):
                        nc.tensor.matmul(
                            psa[:],
                            ztr[n][:, lo:hi],
                            wvr_t[n][:, 0:SQ],
                            start=(n == 0),
                            stop=(n == NDC - 1),
                        )
                    for n in range(NDC):
                        nc.tensor.matmul(
                            psb[:],
                            zti[n][:, lo:hi],
                            wvi_t[n][:, 0:SQ],
                            start=(n == 0),
                            stop=(n == NDC - 1),
                        )
                    nc.scalar.activation(v_tiles[kb][:, 0:SQ], psb[:], COPY, scale=-1.0)
                    nc.vector.tensor_add(
                        v_tiles[kb][:, 0:SQ], psa[:], v_tiles[kb][:, 0:SQ]
                    )
                    ps = pp.tile([P, SQ], f32, tag="pp", name="ps")
                    n = 0
                    for zt, w_t in ((zti, wvr_t), (ztr, wvi_t)):
                        for dc in range(NDC):
                            nc.tensor.matmul(
                                ps[:],
                                zt[dc][:, lo:hi],
                                w_t[dc][:, 0:SQ],
                                start=(n == 0),
                                stop=(n == 15),
                            )
                            n += 1
                    nc.vector.tensor_copy(
                        v_tiles[kb][:, SQ : 2 * SQ], ps[:]
                    )

                # k^T plane loads ride the Scalar engine's HWDGE queue,
                # emitted after this chunk's v copies so their gather wait
                # never head-of-line-blocks the v PSUM consumers.
                for ci in range(2):
                    t = kp.tile([P, NDC * SQ], f16, tag="ktch", name=f"kt{sc}{ci}")
                    load_plane(t, sc, 1, ci, nc.scalar)
                    ktpl[(sc, ci)] = t

    def emit_attention(mask_t, ones_t, v_tiles, ktpl):
        qtt = {}
        with (
            tc.tile_pool(name="qtpl", bufs=2) as qp,
            tc.tile_pool(name="expp", bufs=20) as ep,
            tc.tile_pool(name="outp", bufs=4) as op,
            tc.tile_pool(name="smal", bufs=2) as smp,
            tc.tile_pool(name="spsum", bufs=2, space="PSUM") as sps,
            tc.tile_pool(name="dpsum", bufs=1, space="PSUM") as dps,
            tc.tile_pool(name="avpsum", bufs=5, space="PSUM") as avp,
        ):
            for m in range(NM):
                nkb = 4 * (m + 1)  # causal key blocks for this macro
                jm = 4 * m  # first diagonal key block
                if (m, 0) not in qtt:
                    for ci in range(2):
                        t = qp.tile(
                            [P, NDC * SQ], f16, tag=f"qt{ci}", name=f"qt{m}{ci}"
                        )
                        load_plane(t, m, 0, ci, nc.sync)
                        qtt[(m, ci)] = t
                qt = [qtt[(m, 0)], qtt[(m, 1)]]
                den_ps = dps.tile([P, 64], f32, tag="den", name="den_ps")
                expts = []
                for kb in range(nkb):
                    # diagonal blocks: only queries >= the block's own row
                    # range are causally valid; trim the moving operand.
                    qo = (kb - jm) * P if kb >= jm else 0
                    sc, toff = kb // 4, (kb % 4) * P
                    ps = sps.tile([P, SQ], f32, tag="sc", name="ps")
                    n = 0
                    for ci in range(2):
                        for dc in range(NDC):
                            nc.tensor.matmul(
                                ps[:, qo:SQ],
                                ktpl[(sc, ci)][:, dc * SQ + toff : dc * SQ + toff + P],
                                qt[ci][:, dc * SQ + qo : (dc + 1) * SQ],
                                start=(n == 0),
                                stop=(n == 15),
                            )
                            n += 1
                    et = ep.tile([P, SQ], bf16, tag="et", name="et")
                    nc.scalar.activation(et[:, qo:SQ], ps[:, qo:SQ], EXP, scale=SCALE)
                    if kb >= jm:
                        sub = kb - jm
                        nc.vector.tensor_mul(
                            et[:, sub * P : (sub + 1) * P],
                            et[:, sub * P : (sub + 1) * P],
                            mask_t[sub][:],
                        )
                    expts.append(et)
                    for sub in range(max(0, kb - jm), 4):
                        c = sub * 16 + kb
                        nc.tensor.matmul(
                            den_ps[:, c : c + 1],
                            et[:, sub * P : (sub + 1) * P],
                            ones_t[:],
                            start=True,
                            stop=True,
                        )
                den_sb = smp.tile([P, 4], f32, tag="densb", name="den_sb")
                for sub in range(4):
                    nc.vector.reduce_sum(
                        den_sb[:, sub : sub + 1],
                        den_ps[:, sub * 16 : sub * 16 + jm + sub + 1],
                        axis=AX,
                    )
                recip = smp.tile([P, 4], f32, tag="recip", name="recip")
                nc.vector.reciprocal(recip[:], den_sb[:])

                for pair in range(2):
                    subs = (2 * pair, 2 * pair + 1)
                    j_hi = jm + subs[1]
                    av = {}
                    for sl in range(2):
                        for ci in range(2):
                            av[(sl, ci)] = avp.tile(
                                [P, SQ], f32, tag="av", name=f"av{sl}{ci}"
                            )
                    for kb in range(j_hi + 1):
                        for sl, sub in enumerate(subs):
                            j = jm + sub
                            if kb > j:
                                continue
                            for ci in range(2):
                                nc.tensor.matmul(
                                    av[(sl, ci)][:],
                                    expts[kb][:, sub * P : (sub + 1) * P],
                                    v_tiles[kb][:, ci * SQ : (ci + 1) * SQ],
                                    start=(kb == 0),
                                    stop=(kb == j),
                                )
                    for sl, sub in enumerate(subs):
                        row = m * SQ + sub * P
                        for ci in range(2):
                            ot = op.tile([P, SQ], f32, tag="ot", name="ot")
                            nc.vector.tensor_scalar_mul(
                                ot[:], av[(sl, ci)][:], recip[:, sub : sub + 1]
                            )
                            nc.sync.dma_start(o_d[ci, row : row + P, :], ot[:])

    with tile.TileContext(nc) as tc:
        with tc.tile_pool(name="const", bufs=1) as cp:
            mask_t = []
            for idx in range(4):
                mt = cp.tile([P, P], bf16, tag=f"mask{idx}", name=f"mask{idx}")
                nc.sync.dma_start(mt[:], dmask_d[idx])
                mask_t.append(mt)
            ones_t = cp.tile([P, 1], bf16, tag="ones", name="ones_t")
            nc.vector.memset(ones_t[:], 1.0)
            for _rep in range(reps):
                with (
                    tc.tile_pool(name="vres", bufs=16) as vp,
                    tc.tile_pool(name="ktpl", bufs=8) as kp,
                ):
                    v_tiles = [
                        vp.tile([P, 2 * SQ], f16, tag="v", name=f"v{kb}")
                        for kb in range(NKB)
                    ]
                    ktpl = {}
                    emit_projections(v_tiles, ktpl, kp)
                    emit_attention(mask_t, ones_t, v_tiles, ktpl)

    nc.compile()
    return nc


def get_module(reps: int = 1):
    key = ("nc", reps)
    if key not in _COMPILED:
        _COMPILED[key] = _build_module(reps)
    return _COMPILED[key]


def prepare_in_maps(z_real, z_imag, wq_r, wq_i, wk_r, wk_i, wv_r, wv_i, mask):
    """Host-side sharding/layout prep -> list of per-core input dicts."""
    r = _to_f16
    zT_r = [r(np.asarray(z_real, np.float32)[b].T) for b in range(B)]
    zT_i = [r(np.asarray(z_imag, np.float32)[b].T) for b in range(B)]
    # weights: torch Linear W is [out, in]; matmuls want W^T = [in, out]
    wqr_T = r(np.asarray(wq_r).T)
    wqi_T = r(np.asarray(wq_i).T)
    wkr_T = r(np.asarray(wk_r).T)
    wki_T = r(np.asarray(wk_i).T)
    wvr_T = r(np.asarray(wv_r).T)
    wvi_T = r(np.asarray(wv_i).T)
    # diagonal-block masks from the provided mask (macro 3 as representative)
    mask = np.asarray(mask)
    dmask = np.zeros((4, P, P), np.float32)
    g0 = 3 * SQ
    for idx in range(4):
        k0 = (12 + idx) * P
        q0 = idx * P
        dmask[idx] = (
            mask[g0 + q0 : g0 + q0 + P, k0 : k0 + P] != 0
        ).T.astype(np.float32)
    dmask = _to_bf16(dmask)
    in_maps = []
    for c in range(N_CORES):
        b, dh = c // 2, c % 2
        half = slice(dh * SQ, (dh + 1) * SQ)
        in_maps.append(
            {
                "zr": zT_r[b],
                "zi": zT_i[b],
                "wqr": np.ascontiguousarray(wqr_T[:, half]),
                "wqi": np.ascontiguousarray(wqi_T[:, half]),
                "wkr": np.ascontiguousarray(wkr_T[:, half]),
                "wki": np.ascontiguousarray(wki_T[:, half]),
                "wvr": np.ascontiguousarray(wvr_T[:, half]),
                "wvi": np.ascontiguousarray(wvi_T[:, half]),
                "dmask": dmask,
            }
        )
    return in_maps


def assemble_output(results):
    """Per-core outputs [2, S, 512] -> full [2, B, S, D]."""
    out = np.empty((2, B, S, D), np.float32)
    for c in range(N_CORES):
        b, dh = c // 2, c % 2
        out[:, b, :, dh * SQ : (dh + 1) * SQ] = results[c]["o"]
    return out


def kernel(**inputs) -> np.ndarray:
    from concourse.bass_utils import run_bass_kernel_spmd

    nc = get_module()
    in_maps = prepare_in_maps(**inputs)
    res = run_bass_kernel_spmd(nc, in_maps, core_ids=list(range(N_CORES)))
    return assemble_output(res.results)
